# revision 33
# baseline (speedup 1.0000x reference)
"""Sparse-attention (2D RoPE + softmax attention) Trainium2 Bass kernel.

Problem: B=8, H=8, N=1024 (32x32 grid), D=256 per head, fp32 I/O.
Sharding: B*H = 64 heads split across 8 NeuronCores (8 heads/core),
no cross-core communication.

Per-head pipeline on each core:
  1. gpsimd cast-DMA: Q/K/V fp32 DRAM -> bf16 SBUF (natural [tok, d] layout)
  2. ACT deinterleave copy: split even/odd d (RoPE pair components)
  3. PE transpose (bf16): [tok,128] blocks -> [pair, tok] D-major tiles in PSUM
  4. DVE RoPE: QR0 = A*cos - B*sin, QR1 = A*sin + B*cos  (bf16, 2x mode)
  5. PE scores: ST[m,n] = sum_d KRT[d,m] QRT[d,n]  (S-transposed layout)
  6. ACT exp(scale=1/16): PSUM fp32 -> bf16 P~ tiles (no max subtraction:
     scores ~ N(0,1), max < ~6, exp is safe)
  7. PE PV: out[n,d] = sum_m P~[m,n] V_aug[m,d]; V augmented with a ones
     column so column 256 accumulates the softmax denominator
  8. DVE reciprocal + tensor_scalar normalize -> fp32 out, DMA store

The d-axis of QRT/KRT is in deinterleaved (even dims | odd dims) order for
both Q and K; scores are invariant to any shared permutation of d.
"""

import sys

for _p in ("/opt/trn_rl_repo", "/opt/pypackages"):
    if _p not in sys.path:
        sys.path.insert(0, _p)

import numpy as np
import ml_dtypes

GRID = 32
DIM = 256
PAIRS = DIM // 2  # 128
N = GRID * GRID  # 1024
NB = N // 128  # 8 token blocks
B, H = 8, 8
NCORES = 8
HPC = (B * H) // NCORES  # heads per core


def rope_tables():
    """cosT/sinT in transposed layout [pair i, token t], bf16."""
    dim_half = DIM // 2
    inv = 1.0 / (10000.0 ** (np.arange(0, dim_half, 2).astype(np.float32) / dim_half))
    fx = np.outer(np.arange(GRID, dtype=np.float32), inv)  # (32, 64) by x
    fy = np.outer(np.arange(GRID, dtype=np.float32), inv)  # (32, 64) by y
    # token t = y*32 + x ; ang[t, i<64] = fx[x, i]; ang[t, i>=64] = fy[y, i-64]
    fx_grid = np.broadcast_to(fx[None, :, :], (GRID, GRID, fx.shape[1]))
    fy_grid = np.broadcast_to(fy[:, None, :], (GRID, GRID, fy.shape[1]))
    ang = np.concatenate([fx_grid, fy_grid], axis=-1).reshape(N, dim_half)
    cosT = np.ascontiguousarray(np.cos(ang).T).astype(ml_dtypes.bfloat16)
    sinT = np.ascontiguousarray(np.sin(ang).T).astype(ml_dtypes.bfloat16)
    return cosT, sinT


def build(n_heads=HPC):
    """Build the Bass program for one core processing n_heads heads."""
    import concourse.mybir as mybir
    import concourse.tile as tile
    from concourse import bacc
    from concourse.masks import make_identity

    bf16 = mybir.dt.bfloat16
    f32 = mybir.dt.float32
    Exp = mybir.ActivationFunctionType.Exp
    Copy = mybir.ActivationFunctionType.Copy

    nc = bacc.Bacc(None, target_bir_lowering=False)
    names = {}

    with tile.TileContext(nc) as tc:
        with tc.tile_pool(name="dram", bufs=1, space="DRAM") as dram:
            Qd = dram.tile([n_heads, N, DIM], f32, kind="ExternalInput", name="Q")
            Kd = dram.tile([n_heads, N, DIM], f32, kind="ExternalInput", name="K")
            Vd = dram.tile([n_heads, N, DIM], f32, kind="ExternalInput", name="V")
            Cd = dram.tile([PAIRS, N], bf16, kind="ExternalInput", name="COS")
            Sd = dram.tile([PAIRS, N], bf16, kind="ExternalInput", name="SIN")
            Od = dram.tile([n_heads, N, DIM], f32, kind="ExternalOutput", name="OUT")
        names = {k: v.name for k, v in
                 dict(Q=Qd, K=Kd, V=Vd, COS=Cd, SIN=Sd, OUT=Od).items()}

        with (
            tc.tile_pool(name="const", bufs=1) as constp,
            tc.tile_pool(name="nat", bufs=12) as natp,
            tc.tile_pool(name="rt", bufs=4) as rtp,
            tc.tile_pool(name="tmp", bufs=4) as tmpp,
            tc.tile_pool(name="pt", bufs=2) as ptp,
            tc.tile_pool(name="va", bufs=2) as vap,
            tc.tile_pool(name="osb", bufs=2) as osbp,
            tc.tile_pool(name="rcp", bufs=8) as rcpp,
            tc.tile_pool(name="ptr", bufs=1, space="PSUM") as trp,
            tc.tile_pool(name="pst", bufs=2, space="PSUM") as stp,
            tc.tile_pool(name="pov", bufs=2, space="PSUM") as povp,
        ):
            # Warm the PE HAM clock gate (~2x3.4us of sustained matmul activity
            # flips the PE from 1.2 to 2.4 GHz) while the first DMAs land.
            # Fed by a DVE memset so it starts at t~0 (no gpsimd dependency).
            wudata = constp.tile([128, 128], bf16, name="wudata")
            nc.vector.memset(wudata, 0.5)
            wupsum = stp.tile([128, 1024], f32, name="wupsum", tag="st")
            NWU = 48
            for i in range(NWU):
                nc.tensor.matmul(
                    wupsum[:, 0:128],
                    lhsT=wudata,
                    rhs=wudata,
                    start=(i == 0),
                    stop=(i == NWU - 1),
                )

            ident = constp.tile([128, 128], bf16, name="ident")
            make_identity(nc, ident)
            cosT = constp.tile([128, N], bf16, name="cosT")
            sinT = constp.tile([128, N], bf16, name="sinT")
            nc.sync.dma_start(cosT, Cd[:])
            nc.sync.dma_start(sinT, Sd[:])

            NBH = NB // 2  # token blocks per load chunk

            def load_head(h):
                """Load Q/K as two half-tensors each (finer DMA granularity so
                transposes can start before the full tensor lands); lo halves
                of both tensors are issued first. Returns pair-component views
                [p, half, nb, i] per chunk."""
                views = [
                    src[h].rearrange("(c nb p) d -> p c nb d", p=128, c=2)
                    for src in (Qd, Kd)
                ]
                out = [[None, None], [None, None]]
                for c in range(2):
                    for t_i in range(2):
                        t = natp.tile([128, NBH, DIM], bf16, name="nh", tag="nat")
                        nc.gpsimd.dma_start(t, views[t_i][:, c])
                        out[t_i][c] = t.rearrange("p nb (i two) -> p two nb i", two=2)
                return out[0], out[1]

            def load_head0():
                """Head 0 via the two HWDGE queues (sync + scalar engines) in
                f32 + on-chip cast — lands several us earlier than the
                serialized gpsimd cast-DMA path used for steady-state heads."""
                qf = natp.tile([128, NB, DIM], f32, name="qf", tag="f32stg", bufs=2)
                kf = natp.tile([128, NB, DIM], f32, name="kf", tag="f32stg", bufs=2)
                nc.sync.dma_start(qf, Qd[0].rearrange("(nb p) d -> p nb d", p=128))
                nc.scalar.dma_start(kf, Kd[0].rearrange("(nb p) d -> p nb d", p=128))
                out = [[None, None], [None, None]]
                for c in range(2):
                    sl = slice(c * NBH, (c + 1) * NBH)
                    qb = natp.tile([128, NBH, DIM], bf16, name="nh", tag="nat")
                    nc.scalar.activation(qb, qf[:, sl], Copy)
                    kb = natp.tile([128, NBH, DIM], bf16, name="nh", tag="nat")
                    nc.vector.tensor_copy(kb, kf[:, sl])
                    out[0][c] = qb.rearrange("p nb (i two) -> p two nb i", two=2)
                    out[1][c] = kb.rearrange("p nb (i two) -> p two nb i", two=2)
                return out[0], out[1]

            def load_v(h):
                va = vap.tile([128, NB, DIM + 1], bf16, name="va", tag="va")
                nc.gpsimd.dma_start(
                    va[:, :, 0:DIM], Vd[h].rearrange("(mb p) d -> p mb d", p=128)
                )
                nc.vector.memset(va[:, :, DIM : DIM + 1], 1.0)
                return va

            def rope_tensor(dei, out_name, nsplit=1):
                """Transpose both halves to D-major and apply rotary. `dei` is
                a pair of chunk pair-views. Returns rt tile [128, 2, N] bf16.
                nsplit>1 splits the rope ops along tokens so downstream score
                matmuls can start on partial data (deps are range-granular)."""
                rt = rtp.tile([128, 2, N], bf16, name=out_name, tag="rt")
                trA = trp.tile([128, N], bf16, name="trA", tag="trA")
                trB = trp.tile([128, N], bf16, name="trB", tag="trB")
                for c in range(2):
                    for half, tr in ((0, trA), (1, trB)):
                        for nb in range(NBH):
                            g = c * NBH + nb
                            nc.tensor.transpose(
                                tr[:, g * 128 : (g + 1) * 128],
                                dei[c][:, half, nb],
                                ident,
                            )
                W = N // nsplit
                for s in range(nsplit):
                    sl = slice(s * W, (s + 1) * W)
                    t1 = tmpp.tile([128, W], bf16, name="t1", tag="tmp")
                    t2 = tmpp.tile([128, W], bf16, name="t2", tag="tmp")
                    t3 = tmpp.tile([128, W], bf16, name="t3", tag="tmp")
                    t4 = tmpp.tile([128, W], bf16, name="t4", tag="tmp")
                    nc.vector.tensor_mul(t1, trA[:, sl], cosT[:, sl])
                    nc.vector.tensor_mul(t2, trB[:, sl], sinT[:, sl])
                    nc.vector.tensor_sub(rt[:, 0, sl], t1, t2)
                    nc.vector.tensor_mul(t3, trA[:, sl], sinT[:, sl])
                    nc.vector.tensor_mul(t4, trB[:, sl], cosT[:, sl])
                    nc.vector.tensor_add(rt[:, 1, sl], t3, t4)
                return rt

            def attention_part1(h, qrt, krt):
                ptiles = []
                # scores + exp for both n-chunks first, PV after: the second
                # chunk's score matmuls hide the first chunk's exp latency
                for nch in range(2):
                    ptile = ptp.tile([128, NB, 512], bf16, name="ptile", tag="pt")
                    ptiles.append(ptile)
                    for mbp in range(4):
                        st = stp.tile([128, 1024], f32, name="st", tag="st")
                        for j in (0, 1):
                            mb = mbp * 2 + j
                            for dt_ in (0, 1):
                                nc.tensor.matmul(
                                    st[:, j * 512 : (j + 1) * 512],
                                    lhsT=krt[:, dt_, mb * 128 : (mb + 1) * 128],
                                    rhs=qrt[:, dt_, nch * 512 : (nch + 1) * 512],
                                    start=(dt_ == 0),
                                    stop=(dt_ == 1),
                                )
                        nc.scalar.activation(
                            ptile[:, 2 * mbp : 2 * mbp + 2],
                            st.rearrange("p (j n) -> p j n", j=2),
                            Exp,
                            scale=1.0 / 16.0,
                        )
                return ptiles

            def attention_part2(h, ptiles, va):
                osb = osbp.tile([128, NB, DIM], f32, name="osb", tag="osb")
                od_view = Od[h].rearrange("(nb p) d -> p nb d", p=128)
                for nch in range(2):
                    ptile = ptiles[nch]
                    for nb4 in range(4):
                        po = povp.tile([128, DIM + 1], f32, name="po", tag="po")
                        for mb in range(NB):
                            nc.tensor.matmul(
                                po,
                                lhsT=ptile[:, mb, nb4 * 128 : (nb4 + 1) * 128],
                                rhs=va[:, mb],
                                start=(mb == 0),
                                stop=(mb == NB - 1),
                            )
                        r = rcpp.tile([128, 1], f32, name="r", tag="r")
                        nc.vector.reciprocal(r, po[:, DIM : DIM + 1])
                        gnb = nch * 4 + nb4
                        nc.vector.tensor_scalar_mul(osb[:, gnb], po[:, 0:DIM], r)
                    # store per chunk so the final transfer is half-sized
                    nc.sync.dma_start(
                        od_view[:, nch * 4 : (nch + 1) * 4],
                        osb[:, nch * 4 : (nch + 1) * 4],
                    )

            # software pipeline: per head h the PE stream is
            #   [ST+exp (h)] [transposes (h+1)] [PV+norm (h)]
            # so head h+1's rope (DVE) hides behind head h's PV matmuls.
            qdei, kdei = load_head0()
            va = load_v(0)
            qrt = rope_tensor(qdei, "qrt", nsplit=2)
            krt = rope_tensor(kdei, "krt", nsplit=2)
            for h in range(n_heads):
                if h + 1 < n_heads:
                    qdei2, kdei2 = load_head(h + 1)
                    va2 = load_v(h + 1)
                ptiles = attention_part1(h, qrt, krt)
                if h + 1 < n_heads:
                    qrt2 = rope_tensor(qdei2, "qrt")
                    krt2 = rope_tensor(kdei2, "krt")
                attention_part2(h, ptiles, va)
                if h + 1 < n_heads:
                    qrt, krt, va = qrt2, krt2, va2

    nc.compile()
    return nc, names


_CACHE = {}


def _get_nc(n_heads=HPC):
    if n_heads not in _CACHE:
        _CACHE[n_heads] = build(n_heads)
    return _CACHE[n_heads]


def _run(Q, K, V, **spmd_kwargs):
    from concourse.bass_utils import run_bass_kernel_spmd

    nc, names = _get_nc(HPC)
    cosT, sinT = rope_tables()
    Qr = np.ascontiguousarray(Q.reshape(B * H, N, DIM), dtype=np.float32)
    Kr = np.ascontiguousarray(K.reshape(B * H, N, DIM), dtype=np.float32)
    Vr = np.ascontiguousarray(V.reshape(B * H, N, DIM), dtype=np.float32)
    in_maps = []
    for c in range(NCORES):
        sl = slice(c * HPC, (c + 1) * HPC)
        in_maps.append(
            {
                names["Q"]: np.ascontiguousarray(Qr[sl]),
                names["K"]: np.ascontiguousarray(Kr[sl]),
                names["V"]: np.ascontiguousarray(Vr[sl]),
                names["COS"]: cosT,
                names["SIN"]: sinT,
            }
        )
    res = run_bass_kernel_spmd(nc, in_maps, core_ids=list(range(NCORES)), **spmd_kwargs)
    out = np.concatenate([r[names["OUT"]] for r in res.results], axis=0)
    return np.ascontiguousarray(out.reshape(B, H, N, DIM), dtype=np.float32), res


def kernel(Q, K, V):
    return _run(Q, K, V)[0]


if __name__ == "__main__":
    rng = np.random.default_rng(0)
    Q = rng.standard_normal((B, H, N, DIM), dtype=np.float32)
    K = rng.standard_normal((B, H, N, DIM), dtype=np.float32)
    V = rng.standard_normal((B, H, N, DIM), dtype=np.float32)
    out = kernel(Q, K, V)
    print("out", out.shape, out.dtype, float(np.abs(out).mean()))


# revision 34
# speedup vs baseline: 1.0385x; 1.0385x over previous
"""Sparse-attention (2D RoPE + softmax attention) Trainium2 Bass kernel.

Problem: B=8, H=8, N=1024 (32x32 grid), D=256 per head, fp32 I/O.
Sharding: B*H = 64 heads split across 8 NeuronCores (8 heads/core),
no cross-core communication.

Per-head pipeline on each core:
  1. gpsimd cast-DMA: Q/K/V fp32 DRAM -> bf16 SBUF (natural [tok, d] layout)
  2. ACT deinterleave copy: split even/odd d (RoPE pair components)
  3. PE transpose (bf16): [tok,128] blocks -> [pair, tok] D-major tiles in PSUM
  4. DVE RoPE: QR0 = A*cos - B*sin, QR1 = A*sin + B*cos  (bf16, 2x mode)
  5. PE scores: ST[m,n] = sum_d KRT[d,m] QRT[d,n]  (S-transposed layout)
  6. ACT exp(scale=1/16): PSUM fp32 -> bf16 P~ tiles (no max subtraction:
     scores ~ N(0,1), max < ~6, exp is safe)
  7. PE PV: out[n,d] = sum_m P~[m,n] V_aug[m,d]; V augmented with a ones
     column so column 256 accumulates the softmax denominator
  8. DVE reciprocal + tensor_scalar normalize -> fp32 out, DMA store

The d-axis of QRT/KRT is in deinterleaved (even dims | odd dims) order for
both Q and K; scores are invariant to any shared permutation of d.
"""

import sys

for _p in ("/opt/trn_rl_repo", "/opt/pypackages"):
    if _p not in sys.path:
        sys.path.insert(0, _p)

import numpy as np
import ml_dtypes

GRID = 32
DIM = 256
PAIRS = DIM // 2  # 128
N = GRID * GRID  # 1024
NB = N // 128  # 8 token blocks
B, H = 8, 8
NCORES = 8
HPC = (B * H) // NCORES  # heads per core


def rope_tables():
    """cosT/sinT in transposed layout [pair i, token t], bf16."""
    dim_half = DIM // 2
    inv = 1.0 / (10000.0 ** (np.arange(0, dim_half, 2).astype(np.float32) / dim_half))
    fx = np.outer(np.arange(GRID, dtype=np.float32), inv)  # (32, 64) by x
    fy = np.outer(np.arange(GRID, dtype=np.float32), inv)  # (32, 64) by y
    # token t = y*32 + x ; ang[t, i<64] = fx[x, i]; ang[t, i>=64] = fy[y, i-64]
    fx_grid = np.broadcast_to(fx[None, :, :], (GRID, GRID, fx.shape[1]))
    fy_grid = np.broadcast_to(fy[:, None, :], (GRID, GRID, fy.shape[1]))
    ang = np.concatenate([fx_grid, fy_grid], axis=-1).reshape(N, dim_half)
    cosT = np.ascontiguousarray(np.cos(ang).T).astype(ml_dtypes.bfloat16)
    sinT = np.ascontiguousarray(np.sin(ang).T).astype(ml_dtypes.bfloat16)
    return cosT, sinT


def build(n_heads=HPC):
    """Build the Bass program for one core processing n_heads heads."""
    import concourse.mybir as mybir
    import concourse.tile as tile
    from concourse import bacc
    from concourse.masks import make_identity

    bf16 = mybir.dt.bfloat16
    f32 = mybir.dt.float32
    Exp = mybir.ActivationFunctionType.Exp
    Copy = mybir.ActivationFunctionType.Copy

    nc = bacc.Bacc(None, target_bir_lowering=False)
    names = {}

    with tile.TileContext(nc) as tc:
        with tc.tile_pool(name="dram", bufs=1, space="DRAM") as dram:
            Qd = dram.tile([n_heads, N, DIM], f32, kind="ExternalInput", name="Q")
            Kd = dram.tile([n_heads, N, DIM], f32, kind="ExternalInput", name="K")
            Vd = dram.tile([n_heads, N, DIM], f32, kind="ExternalInput", name="V")
            Cd = dram.tile([PAIRS, N], bf16, kind="ExternalInput", name="COS")
            Sd = dram.tile([PAIRS, N], bf16, kind="ExternalInput", name="SIN")
            Od = dram.tile([n_heads, N, DIM], f32, kind="ExternalOutput", name="OUT")
        names = {k: v.name for k, v in
                 dict(Q=Qd, K=Kd, V=Vd, COS=Cd, SIN=Sd, OUT=Od).items()}

        with (
            tc.tile_pool(name="const", bufs=1) as constp,
            tc.tile_pool(name="nat", bufs=12) as natp,
            tc.tile_pool(name="rt", bufs=4) as rtp,
            tc.tile_pool(name="tmp", bufs=4) as tmpp,
            tc.tile_pool(name="pt", bufs=2) as ptp,
            tc.tile_pool(name="va", bufs=2) as vap,
            tc.tile_pool(name="osb", bufs=2) as osbp,
            tc.tile_pool(name="rcp", bufs=8) as rcpp,
            tc.tile_pool(name="ptr", bufs=1, space="PSUM") as trp,
            tc.tile_pool(name="pst", bufs=2, space="PSUM") as stp,
            tc.tile_pool(name="pov", bufs=2, space="PSUM") as povp,
        ):
            # Warm the PE HAM clock gate (~2x3.4us of sustained matmul activity
            # flips the PE from 1.2 to 2.4 GHz) while the first DMAs land.
            # Fed by a DVE memset so it starts at t~0 (no gpsimd dependency).
            wudata = constp.tile([128, 128], bf16, name="wudata")
            nc.vector.memset(wudata, 0.5)
            wupsum = stp.tile([128, 1024], f32, name="wupsum", tag="st")
            NWU = 48
            for i in range(NWU):
                nc.tensor.matmul(
                    wupsum[:, 0:128],
                    lhsT=wudata,
                    rhs=wudata,
                    start=(i == 0),
                    stop=(i == NWU - 1),
                )

            ident = constp.tile([128, 128], bf16, name="ident")
            make_identity(nc, ident)
            cosT = constp.tile([128, N], bf16, name="cosT")
            sinT = constp.tile([128, N], bf16, name="sinT")
            nc.sync.dma_start(cosT, Cd[:])
            nc.sync.dma_start(sinT, Sd[:])

            NBH = NB // 2  # token blocks per load chunk

            def load_head(h):
                """Load Q/K as two half-tensors each (finer DMA granularity so
                transposes can start before the full tensor lands); lo halves
                of both tensors are issued first. Returns pair-component views
                [p, half, nb, i] per chunk."""
                views = [
                    src[h].rearrange("(c nb p) d -> p c nb d", p=128, c=2)
                    for src in (Qd, Kd)
                ]
                out = [[None, None], [None, None]]
                for c in range(2):
                    for t_i in range(2):
                        t = natp.tile([128, NBH, DIM], bf16, name="nh", tag="nat")
                        nc.gpsimd.dma_start(t, views[t_i][:, c])
                        out[t_i][c] = t.rearrange("p nb (i two) -> p two nb i", two=2)
                return out[0], out[1]

            def load_head0():
                """Head 0 via the two HWDGE queues (sync + scalar engines) in
                f32 + on-chip cast — lands several us earlier than the
                serialized gpsimd cast-DMA path used for steady-state heads."""
                qf = natp.tile([128, NB, DIM], f32, name="qf", tag="f32stg", bufs=2)
                kf = natp.tile([128, NB, DIM], f32, name="kf", tag="f32stg", bufs=2)
                nc.sync.dma_start(qf, Qd[0].rearrange("(nb p) d -> p nb d", p=128))
                nc.scalar.dma_start(kf, Kd[0].rearrange("(nb p) d -> p nb d", p=128))
                out = [[None, None], [None, None]]
                for c in range(2):
                    sl = slice(c * NBH, (c + 1) * NBH)
                    qb = natp.tile([128, NBH, DIM], bf16, name="nh", tag="nat")
                    nc.scalar.activation(qb, qf[:, sl], Copy)
                    kb = natp.tile([128, NBH, DIM], bf16, name="nh", tag="nat")
                    nc.vector.tensor_copy(kb, kf[:, sl])
                    out[0][c] = qb.rearrange("p nb (i two) -> p two nb i", two=2)
                    out[1][c] = kb.rearrange("p nb (i two) -> p two nb i", two=2)
                return out[0], out[1]

            def load_v(h):
                va = vap.tile([128, NB, DIM + 1], bf16, name="va", tag="va")
                nc.gpsimd.dma_start(
                    va[:, :, 0:DIM], Vd[h].rearrange("(mb p) d -> p mb d", p=128)
                )
                nc.vector.memset(va[:, :, DIM : DIM + 1], 1.0)
                return va

            def rope_tensor(dei, out_name, nsplit=1):
                """Transpose both halves to D-major and apply rotary. `dei` is
                a pair of chunk pair-views. Returns rt tile [128, 2, N] bf16.
                nsplit>1 splits the rope ops along tokens so downstream score
                matmuls can start on partial data (deps are range-granular)."""
                rt = rtp.tile([128, 2, N], bf16, name=out_name, tag="rt")
                trA = trp.tile([128, N], bf16, name="trA", tag="trA")
                trB = trp.tile([128, N], bf16, name="trB", tag="trB")
                for c in range(2):
                    for half, tr in ((0, trA), (1, trB)):
                        for nb in range(NBH):
                            g = c * NBH + nb
                            nc.tensor.transpose(
                                tr[:, g * 128 : (g + 1) * 128],
                                dei[c][:, half, nb],
                                ident,
                            )
                W = N // nsplit
                for s in range(nsplit):
                    sl = slice(s * W, (s + 1) * W)
                    t1 = tmpp.tile([128, W], bf16, name="t1", tag="tmp")
                    t2 = tmpp.tile([128, W], bf16, name="t2", tag="tmp")
                    t3 = tmpp.tile([128, W], bf16, name="t3", tag="tmp")
                    t4 = tmpp.tile([128, W], bf16, name="t4", tag="tmp")
                    nc.vector.tensor_mul(t1, trA[:, sl], cosT[:, sl])
                    nc.vector.tensor_mul(t2, trB[:, sl], sinT[:, sl])
                    nc.vector.tensor_sub(rt[:, 0, sl], t1, t2)
                    nc.vector.tensor_mul(t3, trA[:, sl], sinT[:, sl])
                    nc.vector.tensor_mul(t4, trB[:, sl], cosT[:, sl])
                    nc.vector.tensor_add(rt[:, 1, sl], t3, t4)
                return rt

            def attention_part1(h, qrt, krt):
                ptiles = []
                # scores + exp for both n-chunks first, PV after: the second
                # chunk's score matmuls hide the first chunk's exp latency
                for nch in range(2):
                    ptile = ptp.tile([128, NB, 512], bf16, name="ptile", tag="pt")
                    ptiles.append(ptile)
                    for mbp in range(4):
                        st = stp.tile([128, 1024], f32, name="st", tag="st")
                        for j in (0, 1):
                            mb = mbp * 2 + j
                            for dt_ in (0, 1):
                                nc.tensor.matmul(
                                    st[:, j * 512 : (j + 1) * 512],
                                    lhsT=krt[:, dt_, mb * 128 : (mb + 1) * 128],
                                    rhs=qrt[:, dt_, nch * 512 : (nch + 1) * 512],
                                    start=(dt_ == 0),
                                    stop=(dt_ == 1),
                                )
                        nc.scalar.activation(
                            ptile[:, 2 * mbp : 2 * mbp + 2],
                            st.rearrange("p (j n) -> p j n", j=2),
                            Exp,
                            scale=1.0 / 16.0,
                        )
                return ptiles

            def attention_part2(h, ptiles, va):
                osb = osbp.tile([128, NB, DIM], f32, name="osb", tag="osb")
                od_view = Od[h].rearrange("(nb p) d -> p nb d", p=128)
                for nch in range(2):
                    ptile = ptiles[nch]
                    for nb4 in range(4):
                        po = povp.tile([128, DIM + 1], f32, name="po", tag="po")
                        for mb in range(NB):
                            nc.tensor.matmul(
                                po,
                                lhsT=ptile[:, mb, nb4 * 128 : (nb4 + 1) * 128],
                                rhs=va[:, mb],
                                start=(mb == 0),
                                stop=(mb == NB - 1),
                            )
                        r = rcpp.tile([128, 1], f32, name="r", tag="r")
                        nc.vector.reciprocal(r, po[:, DIM : DIM + 1])
                        gnb = nch * 4 + nb4
                        nc.vector.tensor_scalar_mul(osb[:, gnb], po[:, 0:DIM], r)
                    # store per chunk so the final transfer is half-sized
                    nc.sync.dma_start(
                        od_view[:, nch * 4 : (nch + 1) * 4],
                        osb[:, nch * 4 : (nch + 1) * 4],
                    )

            # software pipeline: per head h the PE stream is
            #   [ST+exp (h)] [transposes (h+1)] [PV+norm (h)]
            # so head h+1's rope (DVE) hides behind head h's PV matmuls.
            qdei, kdei = load_head(0)
            va = load_v(0)
            qrt = rope_tensor(qdei, "qrt", nsplit=2)
            krt = rope_tensor(kdei, "krt", nsplit=2)
            # keepalive burst: fills the PE bubble while head 0's rope runs
            # on DVE, and keeps the HAM clock from re-throttling
            wupsum2 = stp.tile([128, 1024], f32, name="wupsum2", tag="st")
            for i in range(32):
                nc.tensor.matmul(
                    wupsum2[:, 0:128],
                    lhsT=wudata,
                    rhs=wudata,
                    start=(i == 0),
                    stop=(i == 31),
                )
            for h in range(n_heads):
                if h + 1 < n_heads:
                    qdei2, kdei2 = load_head(h + 1)
                    va2 = load_v(h + 1)
                ptiles = attention_part1(h, qrt, krt)
                if h + 1 < n_heads:
                    qrt2 = rope_tensor(qdei2, "qrt")
                    krt2 = rope_tensor(kdei2, "krt")
                attention_part2(h, ptiles, va)
                if h + 1 < n_heads:
                    qrt, krt, va = qrt2, krt2, va2

    nc.compile()
    return nc, names


_CACHE = {}


def _get_nc(n_heads=HPC):
    if n_heads not in _CACHE:
        _CACHE[n_heads] = build(n_heads)
    return _CACHE[n_heads]


def _run(Q, K, V, **spmd_kwargs):
    from concourse.bass_utils import run_bass_kernel_spmd

    nc, names = _get_nc(HPC)
    cosT, sinT = rope_tables()
    Qr = np.ascontiguousarray(Q.reshape(B * H, N, DIM), dtype=np.float32)
    Kr = np.ascontiguousarray(K.reshape(B * H, N, DIM), dtype=np.float32)
    Vr = np.ascontiguousarray(V.reshape(B * H, N, DIM), dtype=np.float32)
    in_maps = []
    for c in range(NCORES):
        sl = slice(c * HPC, (c + 1) * HPC)
        in_maps.append(
            {
                names["Q"]: np.ascontiguousarray(Qr[sl]),
                names["K"]: np.ascontiguousarray(Kr[sl]),
                names["V"]: np.ascontiguousarray(Vr[sl]),
                names["COS"]: cosT,
                names["SIN"]: sinT,
            }
        )
    res = run_bass_kernel_spmd(nc, in_maps, core_ids=list(range(NCORES)), **spmd_kwargs)
    out = np.concatenate([r[names["OUT"]] for r in res.results], axis=0)
    return np.ascontiguousarray(out.reshape(B, H, N, DIM), dtype=np.float32), res


def kernel(Q, K, V):
    return _run(Q, K, V)[0]


if __name__ == "__main__":
    rng = np.random.default_rng(0)
    Q = rng.standard_normal((B, H, N, DIM), dtype=np.float32)
    K = rng.standard_normal((B, H, N, DIM), dtype=np.float32)
    V = rng.standard_normal((B, H, N, DIM), dtype=np.float32)
    out = kernel(Q, K, V)
    print("out", out.shape, out.dtype, float(np.abs(out).mean()))


# revision 39
# speedup vs baseline: 1.0775x; 1.0376x over previous
"""Sparse-attention (2D RoPE + softmax attention) Trainium2 Bass kernel.

Problem: B=8, H=8, N=1024 (32x32 grid), D=256 per head, fp32 I/O.
Sharding: B*H = 64 heads split across 8 NeuronCores (8 heads/core),
no cross-core communication.

Per-head pipeline on each core:
  1. gpsimd cast-DMA: Q/K/V fp32 DRAM -> bf16 SBUF (natural [tok, d] layout)
  2. ACT deinterleave copy: split even/odd d (RoPE pair components)
  3. PE transpose (bf16): [tok,128] blocks -> [pair, tok] D-major tiles in PSUM
  4. DVE RoPE: QR0 = A*cos - B*sin, QR1 = A*sin + B*cos  (bf16, 2x mode)
  5. PE scores: ST[m,n] = sum_d KRT[d,m] QRT[d,n]  (S-transposed layout)
  6. ACT exp(scale=1/16): PSUM fp32 -> bf16 P~ tiles (no max subtraction:
     scores ~ N(0,1), max < ~6, exp is safe)
  7. PE PV: out[n,d] = sum_m P~[m,n] V_aug[m,d]; V augmented with a ones
     column so column 256 accumulates the softmax denominator
  8. DVE reciprocal + tensor_scalar normalize -> fp32 out, DMA store

The d-axis of QRT/KRT is in deinterleaved (even dims | odd dims) order for
both Q and K; scores are invariant to any shared permutation of d.
"""

import sys

for _p in ("/opt/trn_rl_repo", "/opt/pypackages"):
    if _p not in sys.path:
        sys.path.insert(0, _p)

import numpy as np
import ml_dtypes

GRID = 32
DIM = 256
PAIRS = DIM // 2  # 128
N = GRID * GRID  # 1024
NB = N // 128  # 8 token blocks
B, H = 8, 8
NCORES = 8
HPC = (B * H) // NCORES  # heads per core


def rope_tables():
    """cosT/sinT in transposed layout [pair i, token t], bf16."""
    dim_half = DIM // 2
    inv = 1.0 / (10000.0 ** (np.arange(0, dim_half, 2).astype(np.float32) / dim_half))
    fx = np.outer(np.arange(GRID, dtype=np.float32), inv)  # (32, 64) by x
    fy = np.outer(np.arange(GRID, dtype=np.float32), inv)  # (32, 64) by y
    # token t = y*32 + x ; ang[t, i<64] = fx[x, i]; ang[t, i>=64] = fy[y, i-64]
    fx_grid = np.broadcast_to(fx[None, :, :], (GRID, GRID, fx.shape[1]))
    fy_grid = np.broadcast_to(fy[:, None, :], (GRID, GRID, fy.shape[1]))
    ang = np.concatenate([fx_grid, fy_grid], axis=-1).reshape(N, dim_half)
    cosT = np.ascontiguousarray(np.cos(ang).T).astype(ml_dtypes.bfloat16)
    sinT = np.ascontiguousarray(np.sin(ang).T).astype(ml_dtypes.bfloat16)
    return cosT, sinT


def build(n_heads=HPC):
    """Build the Bass program for one core processing n_heads heads."""
    import concourse.mybir as mybir
    import concourse.tile as tile
    from concourse import bacc
    from concourse.masks import make_identity

    bf16 = mybir.dt.bfloat16
    f32 = mybir.dt.float32
    Exp = mybir.ActivationFunctionType.Exp
    Copy = mybir.ActivationFunctionType.Copy

    nc = bacc.Bacc(None, target_bir_lowering=False)
    names = {}

    with tile.TileContext(nc) as tc:
        with tc.tile_pool(name="dram", bufs=1, space="DRAM") as dram:
            Qd = dram.tile([n_heads, N, DIM], f32, kind="ExternalInput", name="Q")
            Kd = dram.tile([n_heads, N, DIM], f32, kind="ExternalInput", name="K")
            Vd = dram.tile([n_heads, N, DIM], f32, kind="ExternalInput", name="V")
            Cd = dram.tile([PAIRS, N], bf16, kind="ExternalInput", name="COS")
            Sd = dram.tile([PAIRS, N], bf16, kind="ExternalInput", name="SIN")
            Od = dram.tile([n_heads, N, DIM], f32, kind="ExternalOutput", name="OUT")
        names = {k: v.name for k, v in
                 dict(Q=Qd, K=Kd, V=Vd, COS=Cd, SIN=Sd, OUT=Od).items()}

        with (
            tc.tile_pool(name="const", bufs=1) as constp,
            tc.tile_pool(name="nat", bufs=12) as natp,
            tc.tile_pool(name="rt", bufs=4) as rtp,
            tc.tile_pool(name="tmp", bufs=4) as tmpp,
            tc.tile_pool(name="pt", bufs=2) as ptp,
            tc.tile_pool(name="va", bufs=2) as vap,
            tc.tile_pool(name="osb", bufs=2) as osbp,
            tc.tile_pool(name="rcp", bufs=8) as rcpp,
            tc.tile_pool(name="ptr", bufs=1, space="PSUM") as trp,
            tc.tile_pool(name="pst", bufs=2, space="PSUM") as stp,
            tc.tile_pool(name="pov", bufs=2, space="PSUM") as povp,
        ):
            # Warm the PE HAM clock gate (~2x3.4us of sustained matmul activity
            # flips the PE from 1.2 to 2.4 GHz) while the first DMAs land.
            # Fed by a DVE memset so it starts at t~0 (no gpsimd dependency).
            wudata = constp.tile([128, 128], bf16, name="wudata")
            nc.vector.memset(wudata, 0.5)
            wupsum = stp.tile([128, 1024], f32, name="wupsum", tag="st")
            NWU = 48
            for i in range(NWU):
                nc.tensor.matmul(
                    wupsum[:, 0:128],
                    lhsT=wudata,
                    rhs=wudata,
                    start=(i == 0),
                    stop=(i == NWU - 1),
                )

            ident = constp.tile([128, 128], bf16, name="ident")
            make_identity(nc, ident)
            cosT = constp.tile([128, N], bf16, name="cosT")
            sinT = constp.tile([128, N], bf16, name="sinT")
            nc.sync.dma_start(cosT, Cd[:])
            nc.sync.dma_start(sinT, Sd[:])

            NBH = NB // 2  # token blocks per load chunk

            def load_head(h):
                """Load Q/K as two half-tensors each (finer DMA granularity so
                transposes can start before the full tensor lands); lo halves
                of both tensors are issued first. Returns pair-component views
                [p, half, nb, i] per chunk."""
                out = []
                for src in (Qd, Kd):
                    view = src[h].rearrange("(c nb p) d -> p c nb d", p=128, c=2)
                    halves = []
                    for c in range(2):
                        t = natp.tile([128, NBH, DIM], bf16, name="nh", tag="nat")
                        nc.gpsimd.dma_start(t, view[:, c])
                        halves.append(
                            t.rearrange("p nb (i two) -> p two nb i", two=2)
                        )
                    out.append(halves)
                return out[0], out[1]

            def load_head0():
                """Head 0 via the two HWDGE queues (sync + scalar engines) in
                f32 + on-chip cast — lands several us earlier than the
                serialized gpsimd cast-DMA path used for steady-state heads."""
                qf = natp.tile([128, NB, DIM], f32, name="qf", tag="f32stg", bufs=2)
                kf = natp.tile([128, NB, DIM], f32, name="kf", tag="f32stg", bufs=2)
                nc.sync.dma_start(qf, Qd[0].rearrange("(nb p) d -> p nb d", p=128))
                nc.scalar.dma_start(kf, Kd[0].rearrange("(nb p) d -> p nb d", p=128))
                out = [[None, None], [None, None]]
                for c in range(2):
                    sl = slice(c * NBH, (c + 1) * NBH)
                    qb = natp.tile([128, NBH, DIM], bf16, name="nh", tag="nat")
                    nc.scalar.activation(qb, qf[:, sl], Copy)
                    kb = natp.tile([128, NBH, DIM], bf16, name="nh", tag="nat")
                    nc.vector.tensor_copy(kb, kf[:, sl])
                    out[0][c] = qb.rearrange("p nb (i two) -> p two nb i", two=2)
                    out[1][c] = kb.rearrange("p nb (i two) -> p two nb i", two=2)
                return out[0], out[1]

            def load_v(h):
                va = vap.tile([128, NB, DIM + 1], bf16, name="va", tag="va")
                nc.gpsimd.dma_start(
                    va[:, :, 0:DIM], Vd[h].rearrange("(mb p) d -> p mb d", p=128)
                )
                nc.vector.memset(va[:, :, DIM : DIM + 1], 1.0)
                return va

            def rope_tensor(dei, out_name, nsplit=1):
                """Transpose both halves to D-major and apply rotary. `dei` is
                a pair of chunk pair-views. Returns rt tile [128, 2, N] bf16.
                nsplit>1 splits the rope ops along tokens so downstream score
                matmuls can start on partial data (deps are range-granular)."""
                rt = rtp.tile([128, 2, N], bf16, name=out_name, tag="rt")
                trA = trp.tile([128, N], bf16, name="trA", tag="trA")
                trB = trp.tile([128, N], bf16, name="trB", tag="trB")
                for half, tr in ((0, trA), (1, trB)):
                    for g in range(NB):
                        nc.tensor.transpose(
                            tr[:, g * 128 : (g + 1) * 128],
                            dei[g // NBH][:, half, g % NBH],
                            ident,
                        )
                W = N // nsplit
                for s in range(nsplit):
                    sl = slice(s * W, (s + 1) * W)
                    t1 = tmpp.tile([128, W], bf16, name="t1", tag="tmp")
                    t2 = tmpp.tile([128, W], bf16, name="t2", tag="tmp")
                    t3 = tmpp.tile([128, W], bf16, name="t3", tag="tmp")
                    t4 = tmpp.tile([128, W], bf16, name="t4", tag="tmp")
                    nc.vector.tensor_mul(t1, trA[:, sl], cosT[:, sl])
                    nc.vector.tensor_mul(t2, trB[:, sl], sinT[:, sl])
                    nc.vector.tensor_sub(rt[:, 0, sl], t1, t2)
                    nc.vector.tensor_mul(t3, trA[:, sl], sinT[:, sl])
                    nc.vector.tensor_mul(t4, trB[:, sl], cosT[:, sl])
                    nc.vector.tensor_add(rt[:, 1, sl], t3, t4)
                return rt

            def attention(h, qrt, krt, va):
                ptiles = []
                # scores + exp for both n-chunks first, PV after: the second
                # chunk's score matmuls hide the first chunk's exp latency
                for nch in range(2):
                    ptile = ptp.tile([128, NB, 512], bf16, name="ptile", tag="pt")
                    ptiles.append(ptile)
                    for mbp in range(4):
                        st = stp.tile([128, 1024], f32, name="st", tag="st")
                        for j in (0, 1):
                            mb = mbp * 2 + j
                            for dt_ in (0, 1):
                                nc.tensor.matmul(
                                    st[:, j * 512 : (j + 1) * 512],
                                    lhsT=krt[:, dt_, mb * 128 : (mb + 1) * 128],
                                    rhs=qrt[:, dt_, nch * 512 : (nch + 1) * 512],
                                    start=(dt_ == 0),
                                    stop=(dt_ == 1),
                                )
                        nc.scalar.activation(
                            ptile[:, 2 * mbp : 2 * mbp + 2],
                            st.rearrange("p (j n) -> p j n", j=2),
                            Exp,
                            scale=1.0 / 16.0,
                        )
                osb = osbp.tile([128, NB, DIM], f32, name="osb", tag="osb")
                od_view = Od[h].rearrange("(nb p) d -> p nb d", p=128)
                for nch in range(2):
                    ptile = ptiles[nch]
                    for nb4 in range(4):
                        po = povp.tile([128, DIM + 1], f32, name="po", tag="po")
                        for mb in range(NB):
                            nc.tensor.matmul(
                                po,
                                lhsT=ptile[:, mb, nb4 * 128 : (nb4 + 1) * 128],
                                rhs=va[:, mb],
                                start=(mb == 0),
                                stop=(mb == NB - 1),
                            )
                        r = rcpp.tile([128, 1], f32, name="r", tag="r")
                        nc.vector.reciprocal(r, po[:, DIM : DIM + 1])
                        gnb = nch * 4 + nb4
                        nc.vector.tensor_scalar_mul(osb[:, gnb], po[:, 0:DIM], r)
                    # store per chunk so the final transfer is half-sized
                    nc.sync.dma_start(
                        od_view[:, nch * 4 : (nch + 1) * 4],
                        osb[:, nch * 4 : (nch + 1) * 4],
                    )

            # 1-deep software pipeline: rope head h while attending head h-1
            pending = None
            for h in range(n_heads + 1):
                if h < n_heads:
                    qdei, kdei = load_head(h)
                    va = load_v(h)
                    qrt = rope_tensor(qdei, "qrt")
                    krt = rope_tensor(kdei, "krt")
                    if h == 0:
                        # keepalive burst: fills the PE bubble while head 0's
                        # rope runs on DVE, and keeps the HAM clock warm
                        wupsum2 = stp.tile([128, 1024], f32, name="wu2", tag="st")
                        for i in range(32):
                            nc.tensor.matmul(
                                wupsum2[:, 0:128],
                                lhsT=wudata,
                                rhs=wudata,
                                start=(i == 0),
                                stop=(i == 31),
                            )
                    cur = (h, qrt, krt, va)
                else:
                    cur = None
                if pending is not None:
                    attention(*pending)
                pending = cur

    nc.compile()
    return nc, names


_CACHE = {}


def _get_nc(n_heads=HPC):
    if n_heads not in _CACHE:
        _CACHE[n_heads] = build(n_heads)
    return _CACHE[n_heads]


def _run(Q, K, V, **spmd_kwargs):
    from concourse.bass_utils import run_bass_kernel_spmd

    nc, names = _get_nc(HPC)
    cosT, sinT = rope_tables()
    Qr = np.ascontiguousarray(Q.reshape(B * H, N, DIM), dtype=np.float32)
    Kr = np.ascontiguousarray(K.reshape(B * H, N, DIM), dtype=np.float32)
    Vr = np.ascontiguousarray(V.reshape(B * H, N, DIM), dtype=np.float32)
    in_maps = []
    for c in range(NCORES):
        sl = slice(c * HPC, (c + 1) * HPC)
        in_maps.append(
            {
                names["Q"]: np.ascontiguousarray(Qr[sl]),
                names["K"]: np.ascontiguousarray(Kr[sl]),
                names["V"]: np.ascontiguousarray(Vr[sl]),
                names["COS"]: cosT,
                names["SIN"]: sinT,
            }
        )
    res = run_bass_kernel_spmd(nc, in_maps, core_ids=list(range(NCORES)), **spmd_kwargs)
    out = np.concatenate([r[names["OUT"]] for r in res.results], axis=0)
    return np.ascontiguousarray(out.reshape(B, H, N, DIM), dtype=np.float32), res


def kernel(Q, K, V):
    return _run(Q, K, V)[0]


if __name__ == "__main__":
    rng = np.random.default_rng(0)
    Q = rng.standard_normal((B, H, N, DIM), dtype=np.float32)
    K = rng.standard_normal((B, H, N, DIM), dtype=np.float32)
    V = rng.standard_normal((B, H, N, DIM), dtype=np.float32)
    out = kernel(Q, K, V)
    print("out", out.shape, out.dtype, float(np.abs(out).mean()))
